# revision 15
# baseline (speedup 1.0000x reference)
"""Trainium2 Bass kernel for the DifferentiableLassoSelector problem.

Math (see reference):
  hact = tanh(x[:,:,None]*W1 + b1)          [B, 256, 32]
  Z    = einsum('bnh,nh->bn', hact, W2) + b2 [B, 256]
  Q    = Z^T Z + 1e-4 I ;  p = 0.1*B - Z^T y
  lam  = 500 projected-gradient iterations of the nonneg QP, step 1/lmax
  y_hat = Z @ lam
Returns (y_hat [B], lam [256]).

Sharding: data-parallel over batch across 8 cores. Per-core partial Gram /
correlation reduced with an AllReduce; the tiny 256-dim QP is replicated.

Per-core layout: feature dim on partitions (2 blocks of 128). The tanh is an
ACT op with per-partition scale/bias (W1/b1 columns); the hidden-dim
contraction is a PE matmul against diag(W2[:,h]) accumulated in PSUM over h.
The QP runs only if min(p) < 0 (otherwise lam=0 is the exact fixed point of
the iteration, so skipping reproduces the reference bitwise).
"""

import sys

sys.path.insert(0, "/opt/trn_rl_repo")

from contextlib import ExitStack

import numpy as np
import concourse.bass as bass
import concourse.tile as tile
import concourse.mybir as mybir
from concourse import bacc
from concourse.bass_utils import run_bass_kernel_spmd
from concourse.masks import make_identity

N_CORES = 8
B = 65536
BC = B // N_CORES  # 8192 batch per core
NF = 256  # features
NH = 32  # hidden
ALPHA = 0.1
JITTER = 1e-4
QP_ITERS = 500
POWER_ITERS = 32
F32 = mybir.dt.float32
HALF = BC // 2  # 4096
NCHUNK = BC // 128  # 64 x-chunks of [128, 256]

_cache = {}


def build(with_if=True, with_cc=True, qp_iters=QP_ITERS):
    nc = bacc.Bacc("TRN2", target_bir_lowering=False, debug=False,
                   num_devices=N_CORES)

    x_in = nc.dram_tensor("x", [BC, NF], F32, kind="ExternalInput")
    y_in = nc.dram_tensor("y", [BC, 1], F32, kind="ExternalInput")
    w1_in = nc.dram_tensor("W1", [NF, NH], F32, kind="ExternalInput")
    b1_in = nc.dram_tensor("b1", [NF, NH], F32, kind="ExternalInput")
    w2_in = nc.dram_tensor("W2", [NF, NH], F32, kind="ExternalInput")
    b2_in = nc.dram_tensor("b2", [1, NF], F32, kind="ExternalInput")
    yhat_out = nc.dram_tensor("yhat", [1, BC], F32, kind="ExternalOutput")
    lam_out = nc.dram_tensor("lam", [2, 128], F32, kind="ExternalOutput")

    with tile.TileContext(nc) as tc, ExitStack() as ctx:
        singles = ctx.enter_context(tc.tile_pool(name="singles", bufs=1))
        big = ctx.enter_context(tc.tile_pool(name="big", bufs=1))

        # --- constants / params -------------------------------------------
        ident = singles.tile([128, 128], F32, tag="ident")
        make_identity(nc, ident[:])
        w1 = [singles.tile([128, NH], F32, tag=f"w1_{c}", name=f"w1_{c}") for c in range(2)]
        b1 = [singles.tile([128, NH], F32, tag=f"b1_{c}", name=f"b1_{c}") for c in range(2)]
        w2 = [singles.tile([128, NH], F32, tag=f"w2_{c}", name=f"w2_{c}") for c in range(2)]
        b2 = [singles.tile([128, 1], F32, tag=f"b2_{c}", name=f"b2_{c}") for c in range(2)]
        for c in range(2):
            nc.sync.dma_start(w1[c][:], w1_in[c * 128:(c + 1) * 128, :])
            nc.sync.dma_start(b1[c][:], b1_in[c * 128:(c + 1) * 128, :])
            nc.sync.dma_start(w2[c][:], w2_in[c * 128:(c + 1) * 128, :])
            nc.sync.dma_start(b2[c][:, 0:1], b2_in[0:1, c * 128:(c + 1) * 128])

        # x^T and Z^T, full per-core batch, feature-partition layout
        xT = [big.tile([128, BC], F32, tag=f"xT{c}", name=f"xT{c}") for c in range(2)]
        zT = [big.tile([128, BC], F32, tag=f"zT{c}", name=f"zT{c}") for c in range(2)]

        # --- lead-in: transpose x into feature-partition layout -----------
        with (
            tc.tile_pool(name="xchunk", bufs=4) as xchunk_pool,
            tc.tile_pool(name="tp_ps", bufs=4, space="PSUM") as tp_ps,
        ):
            for t in range(NCHUNK):
                xc = xchunk_pool.tile([128, NF], F32, tag="xc")
                nc.sync.dma_start(xc[:], x_in[t * 128:(t + 1) * 128, :])
                for c in range(2):
                    tp = tp_ps.tile([128, 128], F32, tag="tp")
                    nc.tensor.transpose(tp[:], xc[:, c * 128:(c + 1) * 128],
                                        ident[:])
                    nc.vector.tensor_copy(xT[c][:, t * 128:(t + 1) * 128],
                                          tp[:])

        # --- phase A: Z^T = sum_h diag(W2[:,h]) @ tanh(W1[:,h] x^T + b1) --
        with (
            tc.tile_pool(name="hact", bufs=2) as hact_pool,
            tc.tile_pool(name="diag", bufs=2) as diag_pool,
            tc.tile_pool(name="z_ps", bufs=1, space="PSUM") as z_ps,
        ):
            for c in range(2):
                for half in range(2):
                    zps = [z_ps.tile([128, 512], F32, tag=f"zb{i}", name=f"zb{i}")
                           for i in range(8)]
                    for h in range(NH):
                        dg = diag_pool.tile([128, 128], F32, tag="dg")
                        nc.vector.tensor_scalar_mul(dg[:], ident[:],
                                                    w2[c][:, h:h + 1])
                        ha = hact_pool.tile([128, HALF], F32, tag="ha")
                        nc.scalar.activation(
                            ha[:], xT[c][:, half * HALF:(half + 1) * HALF],
                            mybir.ActivationFunctionType.Tanh,
                            bias=b1[c][:, h:h + 1], scale=w1[c][:, h:h + 1])
                        for i in range(8):
                            nc.tensor.matmul(
                                zps[i][:], dg[:], ha[:, i * 512:(i + 1) * 512],
                                start=(h == 0), stop=(h == NH - 1))
                    for i in range(8):
                        # Z^T = psum + b2 (per-partition scalar add)
                        nc.vector.tensor_scalar_add(
                            zT[c][:, half * HALF + i * 512:
                                  half * HALF + (i + 1) * 512],
                            zps[i][:], b2[c][:, 0:1])

        # --- Gram: Q = Z^T Z (+ Z^T y as column 256) ----------------------
        # Zy chunks [128b, 257]: cols 0:256 = Z rows, col 256 = y
        qps = []
        with (
            tc.tile_pool(name="zsb", bufs=4) as zsb_pool,
            tc.tile_pool(name="tp2_ps", bufs=4, space="PSUM") as tp2_ps,
            tc.tile_pool(name="q_ps", bufs=1, space="PSUM") as q_ps,
        ):
            qps = [q_ps.tile([128, 257], F32, tag=f"q{i}", name=f"q{i}") for i in range(2)]
            for t in range(NCHUNK):
                zsb = zsb_pool.tile([128, 257], F32, tag="zsb")
                for c in range(2):
                    tp = tp2_ps.tile([128, 128], F32, tag="tp2")
                    nc.tensor.transpose(tp[:], zT[c][:, t * 128:(t + 1) * 128],
                                        ident[:])
                    nc.vector.tensor_copy(zsb[:, c * 128:(c + 1) * 128], tp[:])
                nc.sync.dma_start(zsb[:, 256:257],
                                  y_in[t * 128:(t + 1) * 128, :])
                for c in range(2):
                    nc.tensor.matmul(qps[c][:], zsb[:, c * 128:(c + 1) * 128],
                                     zsb[:, 0:257],
                                     start=(t == 0), stop=(t == NCHUNK - 1))

            # --- AllReduce Q/Zty across cores -----------------------------
            qsb_loc = singles.tile([128, 2, 257], F32, tag="qsb_loc")
            for c in range(2):
                nc.vector.tensor_copy(qsb_loc[:, c, :], qps[c][:])

        if with_cc:
            with tc.tile_pool(name="dram", bufs=1, space="DRAM") as dram_pool:
                cc_in = dram_pool.tile([2, 128, 257], F32, tag="cc_in")
                cc_out = dram_pool.tile([2, 128, 257], F32, tag="cc_out")
                for c in range(2):
                    nc.sync.dma_start(cc_in[c], qsb_loc[:, c, :])
                nc.gpsimd.collective_compute(
                    "AllReduce", mybir.AluOpType.add,
                    replica_groups=[list(range(N_CORES))],
                    ins=[cc_in.opt()], outs=[cc_out.opt()])
                qsb = singles.tile([128, 2, 257], F32, tag="qsb")
                for c in range(2):
                    nc.sync.dma_start(qsb[:, c, :], cc_out[c])
        else:
            qsb = qsb_loc

        # add jitter to the diagonal: Q[:, c, c*128+p] += JITTER
        jid = singles.tile([128, 128], F32, tag="jid")
        nc.vector.tensor_scalar_mul(jid[:], ident[:], JITTER)
        for c in range(2):
            nc.vector.tensor_add(qsb[:, c, c * 128:(c + 1) * 128],
                                 qsb[:, c, c * 128:(c + 1) * 128], jid[:])

        # p = ALPHA*B - Zty  (packed [128, 2], col per feature block)
        p_t = singles.tile([128, 2], F32, tag="p")
        for c in range(2):
            # p[:, c] = qsb[:, c, 256] * (-1) + ALPHA*B
            nc.vector.tensor_scalar(p_t[:, c:c + 1], qsb[:, c, 256:257],
                                    -1.0, float(ALPHA * B),
                                    mybir.AluOpType.mult, mybir.AluOpType.add)

        # lam = 0 (the output when the QP is skipped)
        lam = singles.tile([128, 2], F32, tag="lam")
        nc.vector.memset(lam[:], 0.0)

        # convergence-at-zero flag: sum_p max_c relu(-p) == 0  <=>  p >= 0
        negp = singles.tile([128, 2], F32, tag="negp")
        nc.vector.tensor_scalar(negp[:], p_t[:], -1.0, 0.0,
                                mybir.AluOpType.mult, mybir.AluOpType.max)
        negp_m = singles.tile([128, 1], F32, tag="negp_m")
        nc.vector.reduce_max(out=negp_m[:], in_=negp[:],
                             axis=mybir.AxisListType.X)
        ones_col = singles.tile([128, 1], F32, tag="ones_col")
        nc.vector.memset(ones_col[:], 1.0)
        ones_row = singles.tile([1, 128], F32, tag="ones_row")
        nc.vector.memset(ones_row[:], 1.0)
        flag_sb = singles.tile([1, 1], F32, tag="flag")
        with tc.tile_pool(name="f_ps", bufs=1, space="PSUM") as f_ps:
            fp = f_ps.tile([1, 1], F32, tag="fp")
            nc.tensor.matmul(fp[:], ones_col[:], negp_m[:],
                             start=True, stop=True)
            nc.vector.tensor_copy(flag_sb[:], fp[:])

        # raw-bit compare: flag >= 0.0 always, and bits>0 <=> flag > 0.0
        if with_if:
            _, (flag_rv,) = nc.values_load_multi_w_load_instructions(
                flag_sb[0:1, 0:1].bitcast(mybir.dt.int32))

        # --- QP (only when lam=0 is not already the fixed point) ----------
        with (
            tc.tile_pool(name="qp", bufs=1) as qp_pool,
            tc.tile_pool(name="qp_ps", bufs=1, space="PSUM") as qp_ps,
        ):
            if_ctx = tc.If(flag_rv > 0) if with_if else None
            with (if_ctx if if_ctx is not None else ExitStack()):
                # power iteration on Qs = 1e-6*Q for lmax (Rayleigh quotient)
                qs = qp_pool.tile([128, 2, 256], F32, tag="qs")
                nc.vector.tensor_scalar_mul(qs[:], qsb[:, :, 0:256], 1e-6)
                v = qp_pool.tile([128, 2], F32, tag="v")
                nc.vector.memset(v[:], 1.0)
                mv_ps = qp_ps.tile([128, 2], F32, tag="mv")
                for it in range(POWER_ITERS):
                    for ic in range(2):
                        for jc in range(2):
                            nc.tensor.matmul(
                                mv_ps[:, ic:ic + 1],
                                qs[:, jc, ic * 128:(ic + 1) * 128],
                                v[:, jc:jc + 1],
                                start=(jc == 0), stop=(jc == 1))
                    nc.vector.tensor_copy(v[:], mv_ps[:])
                # u = Qs v ; rayleigh = (v.u)/(v.v); lmax = 1e6 * rayleigh
                u = qp_pool.tile([128, 2], F32, tag="u")
                for ic in range(2):
                    for jc in range(2):
                        nc.tensor.matmul(
                            mv_ps[:, ic:ic + 1],
                            qs[:, jc, ic * 128:(ic + 1) * 128],
                            v[:, jc:jc + 1], start=(jc == 0), stop=(jc == 1))
                nc.vector.tensor_copy(u[:], mv_ps[:])
                dots_ps = qp_ps.tile([1, 2], F32, tag="dots")
                for c in range(2):
                    nc.tensor.matmul(dots_ps[:, 0:1], v[:, c:c + 1],
                                     u[:, c:c + 1], start=(c == 0),
                                     stop=(c == 1))
                for c in range(2):
                    nc.tensor.matmul(dots_ps[:, 1:2], v[:, c:c + 1],
                                     v[:, c:c + 1], start=(c == 0),
                                     stop=(c == 1))
                dots = qp_pool.tile([1, 2], F32, tag="dots_sb")
                nc.vector.tensor_copy(dots[:], dots_ps[:])
                # s = (v.v)/(v.u) * 1e-6/1.02   (1/ (1.02*lmax))
                rec = qp_pool.tile([1, 1], F32, tag="rec")
                nc.vector.reciprocal(rec[:], dots[:, 0:1])
                s_sc = qp_pool.tile([1, 1], F32, tag="s_sc")
                nc.vector.tensor_mul(s_sc[:], rec[:], dots[:, 1:2])
                nc.vector.tensor_scalar_mul(s_sc[:], s_sc[:], 1e-6 / 1.02)
                # broadcast s to all partitions via PE (K=1 matmul), negate
                s_ps = qp_ps.tile([128, 1], F32, tag="s_ps")
                nc.tensor.matmul(s_ps[:], ones_row[:], s_sc[:],
                                 start=True, stop=True)
                nsb = qp_pool.tile([128, 1], F32, tag="nsb")
                nc.vector.tensor_scalar_mul(nsb[:], s_ps[:], -1.0)
                sp = qp_pool.tile([128, 2], F32, tag="sp")
                nc.vector.tensor_scalar_mul(sp[:], p_t[:], nsb[:, 0:1])
                # 500 projected-gradient iterations
                y_ps = qp_ps.tile([128, 2], F32, tag="ypg")
                for it in range(qp_iters):
                    for ic in range(2):
                        for jc in range(2):
                            nc.tensor.matmul(
                                y_ps[:, ic:ic + 1],
                                qsb[:, jc, ic * 128:(ic + 1) * 128],
                                lam[:, jc:jc + 1],
                                start=(jc == 0), stop=(jc == 1))
                    upd = qp_pool.tile([128, 2], F32, tag="upd")
                    nc.vector.tensor_scalar(upd[:], y_ps[:], nsb[:, 0:1],
                                            None, mybir.AluOpType.mult)
                    nc.vector.tensor_add(upd[:], upd[:], sp[:])
                    nc.vector.tensor_add(upd[:], upd[:], lam[:])
                    nc.vector.tensor_scalar_max(lam[:], upd[:], 0.0)

        # --- y_hat = Z @ lam ---------------------------------------------
        yh_sb = singles.tile([1, BC], F32, tag="yh")
        with tc.tile_pool(name="yh_ps", bufs=2, space="PSUM") as yh_ps:
            for t in range(BC // 512):
                yp = yh_ps.tile([1, 512], F32, tag="yp")
                for c in range(2):
                    nc.tensor.matmul(yp[:], lam[:, c:c + 1],
                                     zT[c][:, t * 512:(t + 1) * 512],
                                     start=(c == 0), stop=(c == 1))
                nc.scalar.copy(yh_sb[:, t * 512:(t + 1) * 512], yp[:])
        nc.sync.dma_start(yhat_out[:], yh_sb[:])
        for c in range(2):
            nc.sync.dma_start(lam_out[c:c + 1, :], lam[:, c:c + 1])

    nc.compile()
    return nc


def kernel(x, y, W1, b1, W2, b2):
    if "nc" not in _cache:
        _cache["nc"] = build()
    nc = _cache["nc"]
    x = np.ascontiguousarray(np.asarray(x, np.float32))
    y = np.ascontiguousarray(np.asarray(y, np.float32))
    W1 = np.ascontiguousarray(np.asarray(W1, np.float32))
    b1 = np.ascontiguousarray(np.asarray(b1, np.float32))
    W2 = np.ascontiguousarray(np.asarray(W2, np.float32))
    b2 = np.ascontiguousarray(np.asarray(b2, np.float32))
    in_maps = []
    for c in range(N_CORES):
        in_maps.append({
            "x": x[c * BC:(c + 1) * BC],
            "y": y[c * BC:(c + 1) * BC],
            "W1": W1, "b1": b1, "W2": W2, "b2": b2.reshape(1, NF),
        })
    res = run_bass_kernel_spmd(nc, in_maps, core_ids=list(range(N_CORES)))
    outs = res.results
    y_hat = np.concatenate([outs[c]["yhat"][0] for c in range(N_CORES)])
    lam = outs[0]["lam"].reshape(NF)
    return y_hat, lam


if __name__ == "__main__":
    rng = np.random.default_rng(0)
    x = rng.standard_normal((B, NF)).astype(np.float32)
    y = rng.standard_normal((B, 1)).astype(np.float32)
    W1 = rng.uniform(-1, 1, (NF, NH)).astype(np.float32)
    b1 = rng.uniform(-1, 1, (NF, NH)).astype(np.float32)
    W2 = rng.uniform(-0.17, 0.17, (NF, NH)).astype(np.float32)
    b2 = rng.uniform(-0.17, 0.17, NF).astype(np.float32)
    yh, lam = kernel(x=x, y=y, W1=W1, b1=b1, W2=W2, b2=b2)
    print("y_hat", yh.shape, yh.dtype, "absmax", np.abs(yh).max())
    print("lam", lam.shape, lam.dtype, "absmax", np.abs(lam).max())
